# revision 43
# baseline (speedup 1.0000x reference)
"""Trainium2 Bass kernel for nn_MultiHeadAttention_2259152798076 (V3).

Faithful to the reference (source bug included): Q = K = V = x @ W_k.T.
Data-parallel over batch B=8 -> one batch per NeuronCore.

Per-core algorithm (S=2048, 8 heads x d_k=16), with a SYMMETRIC softmax
shift so the P matrix stays transposable:
  h = x @ W_k.T;  m_j = |h_j|^2 / 4
  P''[k,q] = exp(<h_k,h_q>/4 - m_k - m_q)     (symmetric, in (0, 1])
  ctx^T = sum_k (h_k e^{m_k}) P''[k,q];  denom = sum_k e^{m_k} P''[k,q]
  (the e^{m_k} compensation uses the SAME stored fp8 bias values, so the
  key-side shift cancels exactly; the query-side shift is softmax-invariant)
  y = (ctx / denom) @ W_o.T

Speed structure (instruction cost: matmul = out-free-size x cycles/row;
fp8e4+DoubleRow = 0.5 c/r; exp = free-size x 0.83ns, Act engine only):
  * scores: hi-lo fp8 pair h ~= hq + r (both e4m3) in ONE DoubleRow matmul
    per (chunk, head): contraction [32p x 2] computes hq'hq + r'hq + hq'r
    + b_k + b_q (bias rides as extra contraction rows 16-19, split hi/lo
    fp8). 0.5 cycles/row with ~bf16 accuracy.
  * SYMMETRIC MIRRORING: P'' is symmetric, so for score supers (C, I) with
    1 <= C - I <= 2 the exp'd bf16 tiles computed during q-group I are
    PE-transposed (128x128 blocks) and accumulated into ctx[C] immediately;
    q-group C then skips those key-groups. exp tiles drop 256 -> 176
    (Act-engine busy 267us -> ~190us; Act is the hard bottleneck since
    TRN2 runs activations nowhere else and GPSIMD cannot read PSUM).
  * ctx: bf16 (K=128/instr), 32-row head bands, one PSUM bank per q-group,
    3 alive at once (bank tags J mod 3); ones-column -> e^{m_k} gives the
    denominators in the same accumulation.
  * PSUM budget (8 banks): 2x scores [128,1024] + 3x ctx + 1x mirror
    transpose staging; norm/out-proj matmuls reuse the just-freed ctx bank
    (same pool tag) so the scores pipeline keeps both of its slots.
  * scheduling: ctx/mirror jobs run through a depth-4 pending queue and
    norm/out-proj are deferred into the next score stream, so the in-order
    PE queue never stalls behind a pending exp.
  * setup: batched full-tensor DVE/Act/Pool ops + PE transposes build the
    fp8 score operands (element-step-2 fp8 transposes), the bf16 aug ctx
    tensor (g = h * e^m via an indicator-matmul dim-expansion of e^m), and
    the per-head |h|^2 biases (DVE square + segmented reduce).

Layouts:
  SC8[half] fp8 [128, 4*2048] viewed [p, v, s]; head g of half at partitions
  32g..32g+32. v0/v1 = scores lhsT i-slices, v2/v3 = rhs i-slices:
    v0: p<16: hq dims, p>=16: r dims                  (i=0 of lhsT)
    v1: p<16: hq, 1@16, 1@17, bk_hi@18, bk_lo@19      (i=1 of lhsT)
    v2: p<16: hq, p>=16: hq (replicated)              (i=0 of rhs)
    v3: p<16: r, bq_hi@16, bq_lo@17, 1@18, 1@19       (i=1 of rhs)
  Sum over (p, i) = hq'hq + r'hq + hq'r + b_k + b_q.
  vb16all bf16 [128, c(16) x hh(8) x 32]: 16 dims of h*e^m + e^m col + zero
  pads to 32 so every ctx matmul writes full 32-row bands (garbage rows are
  masked by the indicator broadcast in the normalization step).
"""

import numpy as np

B, S, D, H, DK = 8, 2048, 128, 8, 16
NCH = S // 128          # 16 k-chunks of 128
QW = 512
LAM = 3.0               # exp rescale: bias = -(|h|^2 + 4) + 4*LAM
_CACHE = {}

# exp engine balance: modeled ns per [128,1024] activation instruction
_ACT_NS = 1063.0
_POOL_NS = 1517.0


def _build():
    import concourse.bacc as bacc
    import concourse.mybir as mybir
    from concourse import masks
    from concourse.alu_op_type import AluOpType
    from concourse.bass import BassScalarEngine
    from concourse.tile import TileContext

    F32 = mybir.dt.float32
    F32R = mybir.dt.float32r
    BF16 = mybir.dt.bfloat16
    FP8 = mybir.dt.float8e4
    I32 = mybir.dt.int32
    EXP = mybir.ActivationFunctionType.Exp
    DR = mybir.MatmulPerfMode.DoubleRow
    AX_X = mybir.AxisListType.X

    nc = bacc.Bacc("TRN2", target_bir_lowering=False, debug=False, num_devices=8)

    x = nc.dram_tensor("x", [S, D], F32, kind="ExternalInput")
    wk = nc.dram_tensor("wk", [D, D], F32, kind="ExternalInput")
    wo = nc.dram_tensor("wo", [D, D], F32, kind="ExternalInput")
    indg = nc.dram_tensor("indg", [128, 8], F32, kind="ExternalInput")
    indb = nc.dram_tensor("indb", [8, 128], F32, kind="ExternalInput")
    e8d = nc.dram_tensor("e8", [8, 128], F32, kind="ExternalInput")
    y = nc.dram_tensor("y", [S, D], F32, kind="ExternalOutput")

    # deterministic greedy exp balancer: Act reads PSUM directly; Pool cannot
    # access PSUM on TRN2, so Pool-assigned tiles pay a DVE PSUM->SBUF copy.
    eng_t = {"act": 0.0, "pool": 0.0, "dve": 0.0}
    _DVE_CP_NS = 1420.0

    def dve_busy(ns):
        eng_t["dve"] += ns

    with TileContext(nc) as tc:
        with tc.tile_pool(name="persist", bufs=1) as sb:
            ident = sb.tile([128, 128], F32)
            masks.make_identity(nc, ident[:])
            ident8 = sb.tile([128, 128], FP8)
            nc.vector.tensor_copy(ident8[:], ident[:])
            identb = sb.tile([128, 128], BF16)
            nc.vector.tensor_copy(identb[:], ident[:])

            x_sb = sb.tile([128, NCH * 128], F32)
            nc.sync.dma_start(
                out=x_sb[:].rearrange("p (n m) -> p n m", m=128),
                in_=x.rearrange("(n p) m -> p n m", p=128),
            )
            wk_sb = sb.tile([128, 128], F32)
            wo_sb = sb.tile([128, 128], F32)
            indg_sb = sb.tile([128, 8], F32R)
            indb_sb = [sb.tile([4, 128], F32R, name=f"indb{h}") for h in range(2)]
            nc.sync.dma_start(out=wk_sb[:], in_=wk[:])
            nc.sync.dma_start(out=wo_sb[:], in_=wo[:])
            indg_f = sb.tile([128, 8], F32)
            indb_f = [sb.tile([4, 128], F32, name=f"indbf{h}") for h in range(2)]
            nc.sync.dma_start(out=indg_f[:], in_=indg[:])
            nc.vector.tensor_copy(indg_sb[:], indg_f[:])
            for h in range(2):
                nc.sync.dma_start(
                    out=indb_f[h][:], in_=indb[4 * h : 4 * (h + 1), :]
                )
                nc.vector.tensor_copy(indb_sb[h][:], indb_f[h][:])

            wkT = sb.tile([128, 128], F32R)
            xT = sb.tile([128, S], F32R)
            wos = [sb.tile([128, 128], F32, name=f"wos{h}") for h in range(2)]
            woTs = [sb.tile([128, 128], BF16, name=f"woTs{h}") for h in range(2)]

            # fp8 score operand tensors, [p, v(4), s(2048)]
            sc8 = [sb.tile([128, 4 * S], FP8, name=f"sc8_{h}") for h in range(2)]
            # ctx lhsT tensor: single big tensor, chunk-major
            vb16all = sb.tile([128, NCH * 8 * 32], BF16)
            hfull = sb.tile([128, S], F32)               # h natural [p, c, hh, 16]
            v8full = sb.tile([128, S], FP8)
            resfull = sb.tile([128, S], F32)
            vl8full = sb.tile([128, S], FP8)

            def pool_copy(out_ap, in_ap):
                from concourse.bass import BassVectorEngine
                BassVectorEngine.tensor_copy(nc.gpsimd, out_ap, in_ap)

            def act_copy(out_ap, in_ap):
                nc.scalar.activation(
                    out_ap, in_ap, mybir.ActivationFunctionType.Copy, 0.0, 1.0
                )

            with (
                tc.tile_pool(name="initps", bufs=2, space="PSUM") as ips,
                tc.tile_pool(name="initsb", bufs=2) as isb,
                tc.tile_pool(name="in8ps", bufs=2, space="PSUM") as tps,
            ):
                # weight transposes
                tp = ips.tile([128, 512], F32, tag="t")
                nc.tensor.transpose(tp[:, 0:128], wk_sb[:], ident[:])
                nc.vector.tensor_copy(wkT[:], tp[:, 0:128])
                for h in range(2):
                    # spread W_o columns, transpose -> row-spread bf16 W_o.T
                    nc.vector.memset(wos[h][:], 0.0)
                    nc.vector.tensor_copy(
                        wos[h][:].rearrange("p (g c) -> p g c", c=32)[:, :, 0:16],
                        wo_sb[:, 64 * h : 64 * (h + 1)].rearrange(
                            "p (g c) -> p g c", c=16
                        ),
                    )
                    tph = ips.tile([128, 512], F32, tag="t")
                    nc.tensor.transpose(tph[:, 0:128], wos[h][:], ident[:])
                    nc.vector.tensor_copy(woTs[h][:], tph[:, 0:128])

                # xT via PE transposes, 4 chunks per PSUM tile
                for q in range(4):
                    tpn = ips.tile([128, 512], F32, tag="t")
                    for i in range(4):
                        nc.tensor.transpose(
                            tpn[:, 128 * i : 128 * (i + 1)],
                            x_sb[:, 512 * q + 128 * i : 512 * q + 128 * (i + 1)],
                            ident[:],
                        )
                    nc.vector.tensor_copy(xT[:, 512 * q : 512 * (q + 1)], tpn[:])

                # h chunks -> hfull (4 chunks per PSUM tile)
                for q in range(4):
                    hp4 = ips.tile([128, 512], F32, tag="hp")
                    for i in range(4):
                        c = 4 * q + i
                        nc.tensor.matmul(
                            hp4[:, 128 * i : 128 * (i + 1)],
                            xT[:, 128 * c : 128 * (c + 1)],
                            wkT[:],
                            start=True,
                            stop=True,
                        )
                    act_copy(hfull[:, 512 * q : 512 * (q + 1)], hp4[:])

                # batched quantization chains (spread across Act/DVE/Pool)
                act_copy(v8full[:], hfull[:])
                nc.vector.tensor_tensor(
                    resfull[:], hfull[:], v8full[:], AluOpType.subtract
                )
                pool_copy(vl8full[:], resfull[:])

                # bias: b4 = 8 - |h|^2 (LAM=3), split hi/lo fp8; [128, (c hh)]
                hsq = isb.tile([128, S], F32, tag="hsq", bufs=1)
                nc.vector.tensor_tensor(hsq[:], hfull[:], hfull[:], AluOpType.mult)
                hsum = isb.tile([128, 128], F32, tag="hsum", bufs=1)
                nc.vector.tensor_reduce(
                    hsum[:],
                    hsq[:].rearrange("p (ch k) -> p ch k", k=16),
                    AX_X,
                    AluOpType.add,
                )
                b4 = isb.tile([128, 128], F32, tag="b4", bufs=1)
                nc.vector.tensor_scalar(
                    b4[:], hsum[:], -1.0, 0.0, AluOpType.mult,
                    AluOpType.add,
                )
                bhi = isb.tile([128, 128], FP8, tag="bhi", bufs=1)
                bres = isb.tile([128, 128], F32, tag="bres", bufs=1)
                blo = isb.tile([128, 128], FP8, tag="blo", bufs=1)
                act_copy(bhi[:], b4[:])
                nc.vector.tensor_tensor(bres[:], b4[:], bhi[:], AluOpType.subtract)
                pool_copy(blo[:], bres[:])

                # ctx aug tensor: batched zero/ones + data writes
                nc.vector.memset(vb16all[:].bitcast(I32), 0)
                v8w = v8full[:].bitcast(I32).rearrange(
                    "p (c hh w) -> p c hh w", c=NCH, hh=8
                )
                vlw = vl8full[:].bitcast(I32).rearrange(
                    "p (c hh w) -> p c hh w", c=NCH, hh=8
                )

                # interleaved natural tensor [p, c, half, v, slot(128)] + transposes
                itl = isb.tile([128, NCH * 2 * 4 * 128], FP8, tag="itl", bufs=1)
                ilv = itl[:].rearrange(
                    "p (c hf v g k) -> p c hf v g k", c=NCH, hf=2, v=4, g=4
                )
                ilw = itl[:].bitcast(I32).rearrange(
                    "p (c hf v g w) -> p c hf v g w", c=NCH, hf=2, v=4, g=4
                )
                bhiv = bhi[:].rearrange("p (c hh o) -> p c hh o", c=NCH, o=1)
                blov = blo[:].rearrange("p (c hh o) -> p c hh o", c=NCH, o=1)
                for hf in range(2):
                    hs = slice(4 * hf, 4 * (hf + 1))
                    s8 = v8w[:, :, hs, :]
                    sl = vlw[:, :, hs, :]
                    # v0 = [hq; r]
                    nc.vector.tensor_copy(ilw[:, :, hf, 0, :, 0:4], s8)
                    pool_copy(ilw[:, :, hf, 0, :, 4:8], sl)
                    # v1 (lhsT i1) = [hq; 1@16, 1@17, bk_hi@18, bk_lo@19, 0...]
                    nc.vector.tensor_copy(ilw[:, :, hf, 1, :, 0:4], s8)
                    nc.gpsimd.memset(ilw[:, :, hf, 1, :, 4:8], 0)
                    nc.gpsimd.memset(ilv[:, :, hf, 1, :, 16:18], 1.0)
                    nc.vector.tensor_copy(ilv[:, :, hf, 1, :, 18:19], bhiv[:, :, hs, :])
                    nc.vector.tensor_copy(ilv[:, :, hf, 1, :, 19:20], blov[:, :, hs, :])
                    # v2 = [hq; hq]
                    nc.vector.tensor_copy(ilw[:, :, hf, 2, :, 0:4], s8)
                    pool_copy(ilw[:, :, hf, 2, :, 4:8], s8)
                    # v3 (rhs i1) = [r; bq_hi@16, bq_lo@17, 1@18, 1@19, 0...]
                    nc.vector.tensor_copy(ilw[:, :, hf, 3, :, 0:4], sl)
                    nc.gpsimd.memset(ilw[:, :, hf, 3, :, 4:8], 0)
                    nc.vector.tensor_copy(ilv[:, :, hf, 3, :, 16:17], bhiv[:, :, hs, :])
                    nc.vector.tensor_copy(ilv[:, :, hf, 3, :, 17:18], blov[:, :, hs, :])
                    nc.gpsimd.memset(ilv[:, :, hf, 3, :, 18:20], 1.0)

                itf = itl[:].rearrange("p (b k) -> p b k", k=128)  # b = (c hf v)
                for c in range(NCH):
                    for hf in range(2):
                        # fp8 PE transpose writes with element step 2
                        tp8 = tps.tile([128, 1024], FP8, tag="tp8")
                        t2 = tp8[:].rearrange(
                            "p (v k two) -> p v k two", v=4, two=2
                        )
                        for v in range(4):
                            nc.tensor.transpose(
                                t2[:, v, :, 0:1],
                                itf[:, 8 * c + 4 * hf + v, :],
                                ident8[:],
                            )
                        cpf = act_copy if (c + NCH * hf) % 2 else nc.vector.tensor_copy
                        cpf(
                            sc8[hf][:].rearrange("p (v s) -> p v s", v=4)[
                                :, :, 128 * c : 128 * (c + 1)
                            ],
                            t2[:, :, :, 0],
                        )

                # per-key compensation em = exp(-0.25*(bhi+blo)) from the SAME
                # stored fp8 bias values (exact cancellation of the key-side
                # bias baked into every P'' tile); g = h * em (per pos, head)
                bsum = isb.tile([128, 128], F32, tag="bsum", bufs=1)
                nc.vector.tensor_tensor(bsum[:], bhi[:], blo[:], AluOpType.add)
                em = isb.tile([128, 128], F32, tag="em", bufs=1)
                nc.scalar.activation(em[:], bsum[:], EXP, 0.0, -0.25)
                # expand em over the 16 dims of each head: per chunk,
                # em16 = emT.T @ E8  (E8[j, d] = 1 iff d in head j)
                e8f = sb.tile([8, 128], F32)
                nc.sync.dma_start(out=e8f[:], in_=e8d[:])
                e8 = sb.tile([8, 128], F32R)
                nc.vector.tensor_copy(e8[:], e8f[:])
                em16 = sb.tile([128, S], F32)
                for q in range(4):
                    ep4 = ips.tile([128, 512], F32, tag="hp")
                    for i in range(4):
                        c = 4 * q + i
                        emt = ips.tile([8, 128], F32, tag="emt")
                        nc.tensor.transpose(
                            emt[:], em[:, 8 * c : 8 * (c + 1)], ident[:]
                        )
                        emts = isb.tile([8, 128], F32R, tag="emts")
                        nc.vector.tensor_copy(emts[:], emt[:])
                        nc.tensor.matmul(
                            ep4[:, 128 * i : 128 * (i + 1)],
                            emts[:],
                            e8[:],
                            start=True,
                            stop=True,
                        )
                    act_copy(em16[:, 512 * q : 512 * (q + 1)], ep4[:])
                gfull = sb.tile([128, S], F32)
                nc.vector.tensor_tensor(gfull[:], hfull[:], em16[:], AluOpType.mult)
                nc.vector.tensor_copy(
                    vb16all[:].rearrange("p (c hh k) -> p c hh k", c=NCH, hh=8)[
                        :, :, :, 16:17
                    ],
                    em[:].rearrange("p (c hh o) -> p c hh o", c=NCH, o=1),
                )
                act_copy(
                    vb16all[:].rearrange("p (c hh k) -> p c hh k", c=NCH, hh=8)[
                        :, :, :, 0:16
                    ],
                    gfull[:].rearrange("p (c hh k) -> p c hh k", c=NCH, hh=8),
                )

            # ---- main loop (symmetric-P mirroring) ----
            # During qg I we compute score supers (C, I) for key-groups C in
            # the direct set; supers with 1 <= C - I <= 2 are additionally
            # MIRRORED: their exp'd P'' tiles (symmetric, bias -m_k - m_q) are
            # PE-transposed and accumulated into ctx[C] immediately, so qg C
            # skips those key-groups entirely (exp count 256 -> 176 tiles).
            # ctx PSUM: one bank per qg, 3 alive at a time (tags J % 3).
            # norm / out-proj / mirror-transpose PSUM all borrow "s" slots.
            with (
                tc.tile_pool(name="sps", bufs=2, space="PSUM") as sps,
                tc.tile_pool(name="ctxps", bufs=1, space="PSUM") as cps,
                tc.tile_pool(name="mtps", bufs=1, space="PSUM") as mtp,
                tc.tile_pool(name="ptpool", bufs=12) as ptp,
                tc.tile_pool(name="mirsb", bufs=8) as msb,
                tc.tile_pool(name="tailsb", bufs=3) as tsb,
            ):
                def emit_exp(out_ap, in_ap):
                    nc.scalar.activation(out_ap, in_ap, EXP, 0.0, 0.25)

                sc8v = [
                    sc8[half][:].rearrange("p (v s) -> p v s", v=4)
                    for half in range(2)
                ]
                vbv = vb16all[:].rearrange(
                    "p (c w k) -> p c w k", c=NCH, w=8
                )
                # direct key-group sets per qg: C >= J plus unmirrored old
                # groups (J - C > 2); mirrored: 1 <= C - J <= 2
                direct = {
                    J: [C for C in range(4) if C >= J or J - C > 2]
                    for C_ in [0] for J in range(4)
                }
                def run_job(job, ctx_mm_, mirror_job_):
                    if job[0] == "direct":
                        _, I_, g_, hh_, ca, cb, pt_, last_ = job
                        ctx_mm_(I_, g_, hh_, ca, pt_[:, 0:QW], False)
                        ctx_mm_(I_, g_, hh_, cb, pt_[:, QW : 2 * QW], last_)
                    else:
                        _, C_, pts_, rnd_ = job
                        mirror_job_(C_, pts_, rnd_)

                deferred = []
                ctx_bbs = {}
                for half in range(2):
                    ctxb = {}
                    started = set()

                    def get_ctx(J, half=half, ctxb_=None):
                        if J not in ctxb:
                            ctxb[J] = cps.tile(
                                [128, QW], F32, name=f"ctx{half}_{J}",
                                tag=f"ctx{J % 3}",
                            )
                        return ctxb[J]

                    def ctx_mm(J, g, hh, chunk, rhs_ap, last):
                        key = (g, J)
                        st = key not in started
                        started.add(key)
                        nc.tensor.matmul(
                            get_ctx(J)[32 * g : 32 * (g + 1), :],
                            vbv[:, chunk, hh, :],
                            rhs_ap,
                            start=st,
                            stop=last,
                            tile_position=(0, 32 * g),
                            skip_group_check=True,
                        )

                    for I in range(4):
                        q0 = QW * I
                        for g in range(4):
                            half_, g_, I_ = half, g, I
                            hh = 4 * half + g
                            pending = []
                            mir_pend = {}

                            def mirror_job(C, pts, rnd, half=half, g=g, I=I,
                                           hh=hh):
                                # transpose half the pt tiles of super (C, I)
                                # into mirror tiles (keys 4I+b, queries qg C)
                                mt = mtp.tile([128, 1024], BF16, tag="mt")
                                for j in range(4):  # source chunk 4C+j
                                    pt_src = pts[j // 2]
                                    dc = j % 2
                                    for db in range(2):
                                        b = 2 * rnd + db
                                        nc.tensor.transpose(
                                            mt[:, 512 * db + 128 * j :
                                               512 * db + 128 * (j + 1)],
                                            pt_src[:, 512 * dc + 128 * b :
                                                   512 * dc + 128 * (b + 1)],
                                            identb[:],
                                        )
                                mir = msb.tile([128, 1024], BF16, tag="mir")
                                nc.vector.tensor_copy(mir[:], mt[:])
                                for db in range(2):
                                    b = 2 * rnd + db
                                    ctx_mm(
                                        C, g, hh, 4 * I + b,
                                        mir[:, 512 * db : 512 * (db + 1)], False,
                                    )

                            prs = [
                                2 * C + t for C in direct[I] for t in range(2)
                            ]
                            for pi, pr in enumerate(prs):
                                C = pr // 2
                                s_ps = sps.tile([128, 1024], F32, tag="s")
                                for dc in range(2):
                                    c = 2 * pr + dc
                                    nc.tensor.matmul(
                                        s_ps[:, 512 * dc : 512 * (dc + 1)],
                                        sc8v[half][
                                            32 * g : 32 * (g + 1),
                                            0:2,
                                            128 * c : 128 * (c + 1),
                                        ],
                                        sc8v[half][
                                            32 * g : 32 * (g + 1), 2:4,
                                            q0 : q0 + QW,
                                        ],
                                        start=True,
                                        stop=True,
                                        perf_mode=DR,
                                        tile_position=(32 * g, 0),
                                        skip_group_check=True,
                                    )
                                pt = ptp.tile([128, 1024], BF16, tag="ptb")
                                emit_exp(pt[:], s_ps[:])
                                last_direct = pi == len(prs) - 1
                                jobs = [
                                    (
                                        "direct", I, g, hh, 2 * pr, 2 * pr + 1,
                                        pt, last_direct,
                                    )
                                ]
                                if 1 <= C - I <= 2:
                                    mir_pend.setdefault(C, []).append(pt)
                                    if len(mir_pend[C]) == 2:
                                        pts_ = mir_pend.pop(C)
                                        jobs.append(("mirror", C, pts_, 0))
                                        jobs.append(("mirror", C, pts_, 1))
                                for job in jobs:
                                    pending.append(job)
                                while len(pending) > 4:
                                    run_job(pending.pop(0), ctx_mm, mirror_job)
                                if (g or I or pi) and deferred:
                                    deferred.pop(0)()
                            for job in pending:
                                run_job(job, ctx_mm, mirror_job)

                        # qg I fully accumulated (its mirrors arrived earlier)
                        def norm_qg(half=half, I=I, ctx_ps=get_ctx(I)):
                            ctx_sb = tsb.tile(
                                [128, QW], F32R, name=f"cs{half}_{I}",
                                tag=f"cs{I % 2}",
                            )
                            nc.vector.tensor_copy(ctx_sb[:], ctx_ps[:])
                            # norm matmuls reuse the just-freed ctx bank so
                            # the scores pipeline keeps both of its slots
                            nrm = cps.tile(
                                [128, QW], F32, name=f"nrm{half}_{I}",
                                tag=f"ctx{I % 3}",
                            )
                            nc.tensor.matmul(
                                nrm[0:4, :],
                                indg_sb[:, 4 * half : 4 * (half + 1)],
                                ctx_sb[:],
                                start=True,
                                stop=True,
                            )
                            r4 = tsb.tile([4, QW], F32R, tag="r4")
                            with nc.allow_low_precision(
                                reason="f32r output is full fp32 precision"
                            ):
                                nc.vector.reciprocal(r4[:], nrm[0:4, :])
                            nc.tensor.matmul(
                                nrm[:, :], indb_sb[half][:], r4[:],
                                start=True, stop=True,
                            )
                            cbb = tsb.tile(
                                [128, QW], BF16, name=f"cb{half}_{I}",
                                tag=f"cb{half}{I}", bufs=1,
                            )
                            ctx_bbs[(half, I)] = cbb
                            nc.vector.tensor_tensor(
                                cbb[:], ctx_sb[:], nrm[:, :], AluOpType.mult
                            )

                        deferred.append(norm_qg)

                        if half == 1:
                            def out_qg(I=I):
                                for qt in range(QW // 128):
                                    op = cps.tile(
                                        [128, QW], F32, name=f"op{I}_{qt}",
                                        tag=f"ctx{I % 3}",
                                    )
                                    for hf_ in range(2):
                                        nc.tensor.matmul(
                                            op[:, 0:128],
                                            ctx_bbs[(hf_, I)][
                                                :, 128 * qt : 128 * (qt + 1)
                                            ],
                                            woTs[hf_][:],
                                            start=(hf_ == 0),
                                            stop=(hf_ == 1),
                                        )
                                    o_sb = tsb.tile([128, 128], F32, tag="osb")
                                    nc.vector.tensor_copy(o_sb[:], op[:, 0:128])
                                    nc.sync.dma_start(
                                        out=y[
                                            QW * I + 128 * qt :
                                            QW * I + 128 * (qt + 1),
                                            :,
                                        ],
                                        in_=o_sb[:],
                                    )

                            deferred.append(out_qg)
                for job in deferred:
                    job()

    nc.compile()
    return nc


def _host_consts():
    indg = np.zeros((128, 8), np.float32)
    for h in range(2):
        for g in range(4):
            indg[32 * g + 16, 4 * h + g] = 1.0
    indb = np.zeros((8, 128), np.float32)
    for h in range(2):
        for g in range(4):
            indb[4 * h + g, 32 * g : 32 * g + 17] = 1.0
    e8 = np.zeros((8, 128), np.float32)
    for j in range(8):
        e8[j, 16 * j : 16 * (j + 1)] = 1.0
    return indg, indb, e8


def _make_runner(nc):
    """Build the jitted SPMD executable ONCE."""
    import jax
    import numpy as _np
    from jax.sharding import Mesh, PartitionSpec
    from jax.experimental.shard_map import shard_map
    import concourse.mybir as mybir
    from concourse import bass2jax

    bass2jax.install_neuronx_cc_hook()
    in_names, out_names, out_avals = [], [], []
    pname = nc.partition_id_tensor.name if nc.partition_id_tensor else None
    for alloc in nc.m.functions[0].allocations:
        if not isinstance(alloc, mybir.MemoryLocationSet):
            continue
        name = alloc.memorylocations[0].name
        if alloc.kind == "ExternalInput":
            if name != pname:
                in_names.append(name)
        elif alloc.kind == "ExternalOutput":
            out_names.append(name)
            out_avals.append(
                jax.core.ShapedArray(
                    tuple(alloc.tensor_shape), mybir.dt.np(alloc.dtype)
                )
            )
    n_params = len(in_names)
    all_names = in_names + out_names + ([pname] if pname else [])
    zero_shapes = [
        ((B * a.shape[0],) + tuple(a.shape[1:]), a.dtype) for a in out_avals
    ]

    def _body(*args):
        operands = list(args)
        if pname is not None:
            operands.append(bass2jax.partition_id_tensor())
        return tuple(
            bass2jax._bass_exec_p.bind(
                *operands,
                out_avals=tuple(out_avals),
                in_names=tuple(all_names),
                out_names=tuple(out_names),
                lowering_input_output_aliases=(),
                sim_require_finite=True,
                sim_require_nnan=True,
                nc=nc,
            )
        )

    devices = jax.devices()[:B]
    mesh = Mesh(_np.asarray(devices), ("core",))
    donate = tuple(range(n_params, n_params + len(out_names)))
    sharded = jax.jit(
        shard_map(
            _body,
            mesh=mesh,
            in_specs=(PartitionSpec("core"),) * (n_params + len(out_names)),
            out_specs=(PartitionSpec("core"),) * len(out_names),
            check_rep=False,
        ),
        donate_argnums=donate,
        keep_unused=True,
    )

    def run(in_maps):
        concat_in = [
            np.concatenate([np.asarray(m[name]) for m in in_maps], axis=0)
            for name in in_names
        ]
        zeros = [np.zeros(s, d) for s, d in zero_shapes]
        outs = sharded(*concat_in, *zeros)
        yv = np.asarray(outs[out_names.index("y")]).reshape(B, S, D)
        return yv

    return run


def kernel(x, W_k, W_q, W_v, W_o):
    if "nc" not in _CACHE:
        _CACHE["nc"] = _build()
    nc = _CACHE["nc"]

    indg, indb, e8 = _host_consts()
    wk = np.ascontiguousarray(np.asarray(W_k, dtype=np.float32))
    wo = np.ascontiguousarray(np.asarray(W_o, dtype=np.float32))
    xs = np.ascontiguousarray(np.asarray(x, dtype=np.float32))
    in_maps = [
        {"x": xs[b], "wk": wk, "wo": wo, "indg": indg, "indb": indb, "e8": e8}
        for b in range(B)
    ]
    try:
        if "runner" not in _CACHE:
            _CACHE["runner"] = _make_runner(nc)
        return _CACHE["runner"](in_maps)
    except Exception:
        _CACHE.pop("runner", None)
        from concourse.bass_utils import run_bass_kernel_spmd

        res = run_bass_kernel_spmd(nc, in_maps, core_ids=list(range(B)))
        return np.stack([res.results[b]["y"] for b in range(B)], axis=0)


# revision 46
# speedup vs baseline: 1.0007x; 1.0007x over previous
"""Trainium2 Bass kernel for nn_MultiHeadAttention_2259152798076 (V3).

Faithful to the reference (source bug included): Q = K = V = x @ W_k.T.
Data-parallel over batch B=8 -> one batch per NeuronCore.

Per-core algorithm (S=2048, 8 heads x d_k=16), with a SYMMETRIC softmax
shift so the P matrix stays transposable:
  h = x @ W_k.T;  m_j = |h_j|^2 / 4
  P''[k,q] = exp(<h_k,h_q>/4 - m_k - m_q)     (symmetric, in (0, 1])
  ctx^T = sum_k (h_k e^{m_k}) P''[k,q];  denom = sum_k e^{m_k} P''[k,q]
  (the e^{m_k} compensation uses the SAME stored fp8 bias values, so the
  key-side shift cancels exactly; the query-side shift is softmax-invariant)
  y = (ctx / denom) @ W_o.T

Speed structure (instruction cost: matmul = out-free-size x cycles/row;
fp8e4+DoubleRow = 0.5 c/r; exp = free-size x 0.83ns, Act engine only):
  * scores: hi-lo fp8 pair h ~= hq + r (both e4m3) in ONE DoubleRow matmul
    per (chunk, head): contraction [32p x 2] computes hq'hq + r'hq + hq'r
    + b_k + b_q (bias rides as extra contraction rows 16-19, split hi/lo
    fp8). 0.5 cycles/row with ~bf16 accuracy.
  * SYMMETRIC MIRRORING: P'' is symmetric, so for score supers (C, I) with
    1 <= C - I <= 2 the exp'd bf16 tiles computed during q-group I are
    PE-transposed (128x128 blocks) and accumulated into ctx[C] immediately;
    q-group C then skips those key-groups. exp tiles drop 256 -> 176
    (Act-engine busy 267us -> ~190us; Act is the hard bottleneck since
    TRN2 runs activations nowhere else and GPSIMD cannot read PSUM).
  * ctx: bf16 (K=128/instr), 32-row head bands, one PSUM bank per q-group,
    3 alive at once (bank tags J mod 3); ones-column -> e^{m_k} gives the
    denominators in the same accumulation.
  * PSUM budget (8 banks): 2x scores [128,1024] + 3x ctx + 1x mirror
    transpose staging; norm/out-proj matmuls reuse the just-freed ctx bank
    (same pool tag) so the scores pipeline keeps both of its slots.
  * scheduling: ctx/mirror jobs run through a depth-4 pending queue and
    norm/out-proj are deferred into the next score stream, so the in-order
    PE queue never stalls behind a pending exp.
  * setup: batched full-tensor DVE/Act/Pool ops + PE transposes build the
    fp8 score operands (element-step-2 fp8 transposes), the bf16 aug ctx
    tensor (g = h * e^m via an indicator-matmul dim-expansion of e^m), and
    the per-head |h|^2 biases (DVE square + segmented reduce).

Layouts:
  SC8[half] fp8 [128, 4*2048] viewed [p, v, s]; head g of half at partitions
  32g..32g+32. v0/v1 = scores lhsT i-slices, v2/v3 = rhs i-slices:
    v0: p<16: hq dims, p>=16: r dims                  (i=0 of lhsT)
    v1: p<16: hq, 1@16, 1@17, bk_hi@18, bk_lo@19      (i=1 of lhsT)
    v2: p<16: hq, p>=16: hq (replicated)              (i=0 of rhs)
    v3: p<16: r, bq_hi@16, bq_lo@17, 1@18, 1@19       (i=1 of rhs)
  Sum over (p, i) = hq'hq + r'hq + hq'r + b_k + b_q.
  vb16all bf16 [128, c(16) x hh(8) x 32]: 16 dims of h*e^m + e^m col + zero
  pads to 32 so every ctx matmul writes full 32-row bands (garbage rows are
  masked by the indicator broadcast in the normalization step).
"""

import numpy as np

B, S, D, H, DK = 8, 2048, 128, 8, 16
NCH = S // 128          # 16 k-chunks of 128
QW = 512
LAM = 3.0               # exp rescale: bias = -(|h|^2 + 4) + 4*LAM
_CACHE = {}

# exp engine balance: modeled ns per [128,1024] activation instruction
_ACT_NS = 1063.0
_POOL_NS = 1517.0


def _build():
    import concourse.bacc as bacc
    import concourse.mybir as mybir
    from concourse import masks
    from concourse.alu_op_type import AluOpType
    from concourse.bass import BassScalarEngine
    from concourse.tile import TileContext

    F32 = mybir.dt.float32
    F32R = mybir.dt.float32r
    BF16 = mybir.dt.bfloat16
    FP8 = mybir.dt.float8e4
    I32 = mybir.dt.int32
    EXP = mybir.ActivationFunctionType.Exp
    DR = mybir.MatmulPerfMode.DoubleRow
    AX_X = mybir.AxisListType.X

    nc = bacc.Bacc("TRN2", target_bir_lowering=False, debug=False, num_devices=8)

    x = nc.dram_tensor("x", [S, D], F32, kind="ExternalInput")
    wk = nc.dram_tensor("wk", [D, D], F32, kind="ExternalInput")
    wo = nc.dram_tensor("wo", [D, D], F32, kind="ExternalInput")
    indg = nc.dram_tensor("indg", [128, 8], F32, kind="ExternalInput")
    indb = nc.dram_tensor("indb", [8, 128], F32, kind="ExternalInput")
    e8d = nc.dram_tensor("e8", [8, 128], F32, kind="ExternalInput")
    y = nc.dram_tensor("y", [S, D], F32, kind="ExternalOutput")

    # deterministic greedy exp balancer: Act reads PSUM directly; Pool cannot
    # access PSUM on TRN2, so Pool-assigned tiles pay a DVE PSUM->SBUF copy.
    eng_t = {"act": 0.0, "pool": 0.0, "dve": 0.0}
    _DVE_CP_NS = 1420.0

    def dve_busy(ns):
        eng_t["dve"] += ns

    with TileContext(nc) as tc:
        with tc.tile_pool(name="persist", bufs=1) as sb:
            ident = sb.tile([128, 128], F32)
            masks.make_identity(nc, ident[:])
            ident8 = sb.tile([128, 128], FP8)
            nc.vector.tensor_copy(ident8[:], ident[:])
            identb = sb.tile([128, 128], BF16)
            nc.vector.tensor_copy(identb[:], ident[:])

            x_sb = sb.tile([128, NCH * 128], F32)
            nc.sync.dma_start(
                out=x_sb[:].rearrange("p (n m) -> p n m", m=128),
                in_=x.rearrange("(n p) m -> p n m", p=128),
            )
            wk_sb = sb.tile([128, 128], F32)
            wo_sb = sb.tile([128, 128], F32)
            indg_sb = sb.tile([128, 8], F32R)
            indb_sb = [sb.tile([4, 128], F32R, name=f"indb{h}") for h in range(2)]
            nc.sync.dma_start(out=wk_sb[:], in_=wk[:])
            nc.sync.dma_start(out=wo_sb[:], in_=wo[:])
            indg_f = sb.tile([128, 8], F32)
            indb_f = [sb.tile([4, 128], F32, name=f"indbf{h}") for h in range(2)]
            nc.sync.dma_start(out=indg_f[:], in_=indg[:])
            nc.vector.tensor_copy(indg_sb[:], indg_f[:])
            for h in range(2):
                nc.sync.dma_start(
                    out=indb_f[h][:], in_=indb[4 * h : 4 * (h + 1), :]
                )
                nc.vector.tensor_copy(indb_sb[h][:], indb_f[h][:])

            wkT = sb.tile([128, 128], F32R)
            xT = sb.tile([128, S], F32R)
            wos = [sb.tile([128, 128], F32, name=f"wos{h}") for h in range(2)]
            woTs = [sb.tile([128, 128], BF16, name=f"woTs{h}") for h in range(2)]

            # fp8 score operand tensors, [p, v(4), s(2048)]
            sc8 = [sb.tile([128, 4 * S], FP8, name=f"sc8_{h}") for h in range(2)]
            # ctx lhsT tensor: single big tensor, chunk-major
            vb16all = sb.tile([128, NCH * 8 * 32], BF16)
            hfull = sb.tile([128, S], F32)               # h natural [p, c, hh, 16]
            v8full = sb.tile([128, S], FP8)
            resfull = sb.tile([128, S], F32)
            vl8full = sb.tile([128, S], FP8)

            def pool_copy(out_ap, in_ap):
                from concourse.bass import BassVectorEngine
                BassVectorEngine.tensor_copy(nc.gpsimd, out_ap, in_ap)

            def act_copy(out_ap, in_ap):
                nc.scalar.activation(
                    out_ap, in_ap, mybir.ActivationFunctionType.Copy, 0.0, 1.0
                )

            with (
                tc.tile_pool(name="initps", bufs=2, space="PSUM") as ips,
                tc.tile_pool(name="initsb", bufs=2) as isb,
                tc.tile_pool(name="in8ps", bufs=2, space="PSUM") as tps,
            ):
                # weight transposes
                tp = ips.tile([128, 512], F32, tag="t")
                nc.tensor.transpose(tp[:, 0:128], wk_sb[:], ident[:])
                nc.vector.tensor_copy(wkT[:], tp[:, 0:128])
                for h in range(2):
                    # spread W_o columns, transpose -> row-spread bf16 W_o.T
                    nc.vector.memset(wos[h][:], 0.0)
                    nc.vector.tensor_copy(
                        wos[h][:].rearrange("p (g c) -> p g c", c=32)[:, :, 0:16],
                        wo_sb[:, 64 * h : 64 * (h + 1)].rearrange(
                            "p (g c) -> p g c", c=16
                        ),
                    )
                    tph = ips.tile([128, 512], F32, tag="t")
                    nc.tensor.transpose(tph[:, 0:128], wos[h][:], ident[:])
                    nc.vector.tensor_copy(woTs[h][:], tph[:, 0:128])

                # xT via PE transposes, 4 chunks per PSUM tile
                for q in range(4):
                    tpn = ips.tile([128, 512], F32, tag="t")
                    for i in range(4):
                        nc.tensor.transpose(
                            tpn[:, 128 * i : 128 * (i + 1)],
                            x_sb[:, 512 * q + 128 * i : 512 * q + 128 * (i + 1)],
                            ident[:],
                        )
                    nc.vector.tensor_copy(xT[:, 512 * q : 512 * (q + 1)], tpn[:])

                # h chunks -> hfull (4 chunks per PSUM tile)
                for q in range(4):
                    hp4 = ips.tile([128, 512], F32, tag="hp")
                    for i in range(4):
                        c = 4 * q + i
                        nc.tensor.matmul(
                            hp4[:, 128 * i : 128 * (i + 1)],
                            xT[:, 128 * c : 128 * (c + 1)],
                            wkT[:],
                            start=True,
                            stop=True,
                        )
                    act_copy(hfull[:, 512 * q : 512 * (q + 1)], hp4[:])

                # batched quantization chains (spread across Act/DVE/Pool)
                act_copy(v8full[:], hfull[:])
                nc.vector.tensor_tensor(
                    resfull[:], hfull[:], v8full[:], AluOpType.subtract
                )
                pool_copy(vl8full[:], resfull[:])

                # bias: b4 = 8 - |h|^2 (LAM=3), split hi/lo fp8; [128, (c hh)]
                hsq = isb.tile([128, S], F32, tag="hsq", bufs=1)
                nc.vector.tensor_tensor(hsq[:], hfull[:], hfull[:], AluOpType.mult)
                hsum = isb.tile([128, 128], F32, tag="hsum", bufs=1)
                nc.vector.tensor_reduce(
                    hsum[:],
                    hsq[:].rearrange("p (ch k) -> p ch k", k=16),
                    AX_X,
                    AluOpType.add,
                )
                b4 = isb.tile([128, 128], F32, tag="b4", bufs=1)
                nc.vector.tensor_scalar(
                    b4[:], hsum[:], -1.0, 0.0, AluOpType.mult,
                    AluOpType.add,
                )
                bhi = isb.tile([128, 128], FP8, tag="bhi", bufs=1)
                bres = isb.tile([128, 128], F32, tag="bres", bufs=1)
                blo = isb.tile([128, 128], FP8, tag="blo", bufs=1)
                act_copy(bhi[:], b4[:])
                nc.vector.tensor_tensor(bres[:], b4[:], bhi[:], AluOpType.subtract)
                pool_copy(blo[:], bres[:])

                # ctx aug tensor: batched zero/ones + data writes
                nc.vector.memset(vb16all[:].bitcast(I32), 0)
                v8w = v8full[:].bitcast(I32).rearrange(
                    "p (c hh w) -> p c hh w", c=NCH, hh=8
                )
                vlw = vl8full[:].bitcast(I32).rearrange(
                    "p (c hh w) -> p c hh w", c=NCH, hh=8
                )

                # interleaved natural tensor [p, c, half, v, slot(128)] + transposes
                itl = isb.tile([128, NCH * 2 * 4 * 128], FP8, tag="itl", bufs=1)
                ilv = itl[:].rearrange(
                    "p (c hf v g k) -> p c hf v g k", c=NCH, hf=2, v=4, g=4
                )
                ilw = itl[:].bitcast(I32).rearrange(
                    "p (c hf v g w) -> p c hf v g w", c=NCH, hf=2, v=4, g=4
                )
                bhiv = bhi[:].rearrange("p (c hh o) -> p c hh o", c=NCH, o=1)
                blov = blo[:].rearrange("p (c hh o) -> p c hh o", c=NCH, o=1)
                for hf in range(2):
                    hs = slice(4 * hf, 4 * (hf + 1))
                    s8 = v8w[:, :, hs, :]
                    sl = vlw[:, :, hs, :]
                    # v0 = [hq; r]
                    nc.vector.tensor_copy(ilw[:, :, hf, 0, :, 0:4], s8)
                    pool_copy(ilw[:, :, hf, 0, :, 4:8], sl)
                    # v1 (lhsT i1) = [hq; 1@16, 1@17, bk_hi@18, bk_lo@19, 0...]
                    nc.vector.tensor_copy(ilw[:, :, hf, 1, :, 0:4], s8)
                    nc.gpsimd.memset(ilw[:, :, hf, 1, :, 4:8], 0)
                    nc.gpsimd.memset(ilv[:, :, hf, 1, :, 16:18], 1.0)
                    nc.vector.tensor_copy(ilv[:, :, hf, 1, :, 18:19], bhiv[:, :, hs, :])
                    nc.vector.tensor_copy(ilv[:, :, hf, 1, :, 19:20], blov[:, :, hs, :])
                    # v2 = [hq; hq]
                    nc.vector.tensor_copy(ilw[:, :, hf, 2, :, 0:4], s8)
                    pool_copy(ilw[:, :, hf, 2, :, 4:8], s8)
                    # v3 (rhs i1) = [r; bq_hi@16, bq_lo@17, 1@18, 1@19, 0...]
                    nc.vector.tensor_copy(ilw[:, :, hf, 3, :, 0:4], sl)
                    nc.gpsimd.memset(ilw[:, :, hf, 3, :, 4:8], 0)
                    nc.vector.tensor_copy(ilv[:, :, hf, 3, :, 16:17], bhiv[:, :, hs, :])
                    nc.vector.tensor_copy(ilv[:, :, hf, 3, :, 17:18], blov[:, :, hs, :])
                    nc.gpsimd.memset(ilv[:, :, hf, 3, :, 18:20], 1.0)

                itf = itl[:].rearrange("p (b k) -> p b k", k=128)  # b = (c hf v)
                for c in range(NCH):
                    for hf in range(2):
                        # fp8 PE transpose writes with element step 2
                        tp8 = tps.tile([128, 1024], FP8, tag="tp8")
                        t2 = tp8[:].rearrange(
                            "p (v k two) -> p v k two", v=4, two=2
                        )
                        for v in range(4):
                            nc.tensor.transpose(
                                t2[:, v, :, 0:1],
                                itf[:, 8 * c + 4 * hf + v, :],
                                ident8[:],
                            )
                        cpf = act_copy if (c + NCH * hf) % 2 else nc.vector.tensor_copy
                        cpf(
                            sc8[hf][:].rearrange("p (v s) -> p v s", v=4)[
                                :, :, 128 * c : 128 * (c + 1)
                            ],
                            t2[:, :, :, 0],
                        )

                # per-key compensation em = exp(-0.25*(bhi+blo)) from the SAME
                # stored fp8 bias values (exact cancellation of the key-side
                # bias baked into every P'' tile); g = h * em (per pos, head)
                bsum = isb.tile([128, 128], F32, tag="bsum", bufs=1)
                nc.vector.tensor_tensor(bsum[:], bhi[:], blo[:], AluOpType.add)
                em = isb.tile([128, 128], F32, tag="em", bufs=1)
                nc.scalar.activation(em[:], bsum[:], EXP, 0.0, -0.25)
                # expand em over the 16 dims of each head: per chunk,
                # em16 = emT.T @ E8  (E8[j, d] = 1 iff d in head j)
                e8f = sb.tile([8, 128], F32)
                nc.sync.dma_start(out=e8f[:], in_=e8d[:])
                e8 = sb.tile([8, 128], F32R)
                nc.vector.tensor_copy(e8[:], e8f[:])
                em16 = sb.tile([128, S], F32)
                for q in range(4):
                    ep4 = ips.tile([128, 512], F32, tag="hp")
                    for i in range(4):
                        c = 4 * q + i
                        emt = ips.tile([8, 128], F32, tag="emt")
                        nc.tensor.transpose(
                            emt[:], em[:, 8 * c : 8 * (c + 1)], ident[:]
                        )
                        emts = isb.tile([8, 128], F32R, tag="emts")
                        nc.vector.tensor_copy(emts[:], emt[:])
                        nc.tensor.matmul(
                            ep4[:, 128 * i : 128 * (i + 1)],
                            emts[:],
                            e8[:],
                            start=True,
                            stop=True,
                        )
                    act_copy(em16[:, 512 * q : 512 * (q + 1)], ep4[:])
                gfull = sb.tile([128, S], F32)
                nc.vector.tensor_tensor(gfull[:], hfull[:], em16[:], AluOpType.mult)
                nc.vector.tensor_copy(
                    vb16all[:].rearrange("p (c hh k) -> p c hh k", c=NCH, hh=8)[
                        :, :, :, 16:17
                    ],
                    em[:].rearrange("p (c hh o) -> p c hh o", c=NCH, o=1),
                )
                act_copy(
                    vb16all[:].rearrange("p (c hh k) -> p c hh k", c=NCH, hh=8)[
                        :, :, :, 0:16
                    ],
                    gfull[:].rearrange("p (c hh k) -> p c hh k", c=NCH, hh=8),
                )

            # ---- main loop (symmetric-P mirroring) ----
            # During qg I we compute score supers (C, I) for key-groups C in
            # the direct set; supers with 1 <= C - I <= 2 are additionally
            # MIRRORED: their exp'd P'' tiles (symmetric, bias -m_k - m_q) are
            # PE-transposed and accumulated into ctx[C] immediately, so qg C
            # skips those key-groups entirely (exp count 256 -> 176 tiles).
            # ctx PSUM: one bank per qg, 3 alive at a time (tags J % 3).
            # norm / out-proj / mirror-transpose PSUM all borrow "s" slots.
            with (
                tc.tile_pool(name="sps", bufs=2, space="PSUM") as sps,
                tc.tile_pool(name="ctxps", bufs=1, space="PSUM") as cps,
                tc.tile_pool(name="mtps", bufs=1, space="PSUM") as mtp,
                tc.tile_pool(name="ptpool", bufs=12) as ptp,
                tc.tile_pool(name="mirsb", bufs=8) as msb,
                tc.tile_pool(name="tailsb", bufs=3) as tsb,
            ):
                def emit_exp(out_ap, in_ap):
                    nc.scalar.activation(out_ap, in_ap, EXP, 0.0, 0.25)

                sc8v = [
                    sc8[half][:].rearrange("p (v s) -> p v s", v=4)
                    for half in range(2)
                ]
                vbv = vb16all[:].rearrange(
                    "p (c w k) -> p c w k", c=NCH, w=8
                )
                # direct key-group sets per qg: C >= J plus unmirrored old
                # groups (J - C > 2); mirrored: 1 <= C - J <= 2
                direct = {
                    J: [C for C in range(4) if C >= J or J - C > 2]
                    for C_ in [0] for J in range(4)
                }
                def run_job(job, ctx_mm_, mirror_job_):
                    if job[0] == "direct":
                        _, I_, g_, hh_, ca, cb, pt_, last_ = job
                        ctx_mm_(I_, g_, hh_, ca, pt_[:, 0:QW], False)
                        ctx_mm_(I_, g_, hh_, cb, pt_[:, QW : 2 * QW], last_)
                    else:
                        _, C_, pts_, rnd_ = job
                        mirror_job_(C_, pts_, rnd_)

                deferred = []
                ctx_bbs = {}
                for half in range(2):
                    ctxb = {}
                    started = set()

                    def get_ctx(J, half=half, ctxb_=None):
                        if J not in ctxb:
                            ctxb[J] = cps.tile(
                                [128, QW], F32, name=f"ctx{half}_{J}",
                                tag=f"ctx{J % 3}",
                            )
                        return ctxb[J]

                    def ctx_mm(J, g, hh, chunk, rhs_ap, last):
                        key = (g, J)
                        st = key not in started
                        started.add(key)
                        nc.tensor.matmul(
                            get_ctx(J)[32 * g : 32 * (g + 1), :],
                            vbv[:, chunk, hh, :],
                            rhs_ap,
                            start=st,
                            stop=last,
                            tile_position=(0, 32 * g),
                            skip_group_check=True,
                        )

                    for I in range(4):
                        q0 = QW * I
                        for g in range(4):
                            half_, g_, I_ = half, g, I
                            hh = 4 * half + g
                            pending = []
                            mir_pend = {}

                            def mirror_job(C, pts, rnd, half=half, g=g, I=I,
                                           hh=hh):
                                # transpose half the pt tiles of super (C, I)
                                # into mirror tiles (keys 4I+b, queries qg C)
                                mt = mtp.tile([128, 1024], BF16, tag="mt")
                                for j in range(4):  # source chunk 4C+j
                                    pt_src = pts[j // 2]
                                    dc = j % 2
                                    for db in range(2):
                                        b = 2 * rnd + db
                                        nc.tensor.transpose(
                                            mt[:, 512 * db + 128 * j :
                                               512 * db + 128 * (j + 1)],
                                            pt_src[:, 512 * dc + 128 * b :
                                                   512 * dc + 128 * (b + 1)],
                                            identb[:],
                                        )
                                mir = msb.tile([128, 1024], BF16, tag="mir")
                                nc.vector.tensor_copy(mir[:], mt[:])
                                for db in range(2):
                                    b = 2 * rnd + db
                                    ctx_mm(
                                        C, g, hh, 4 * I + b,
                                        mir[:, 512 * db : 512 * (db + 1)], False,
                                    )

                            prs = [
                                2 * C + t for C in direct[I] for t in range(2)
                            ]
                            for pi, pr in enumerate(prs):
                                C = pr // 2
                                s_ps = sps.tile([128, 1024], F32, tag="s")
                                for dc in range(2):
                                    c = 2 * pr + dc
                                    nc.tensor.matmul(
                                        s_ps[:, 512 * dc : 512 * (dc + 1)],
                                        sc8v[half][
                                            32 * g : 32 * (g + 1),
                                            0:2,
                                            128 * c : 128 * (c + 1),
                                        ],
                                        sc8v[half][
                                            32 * g : 32 * (g + 1), 2:4,
                                            q0 : q0 + QW,
                                        ],
                                        start=True,
                                        stop=True,
                                        perf_mode=DR,
                                        tile_position=(32 * g, 0),
                                        skip_group_check=True,
                                    )
                                pt = ptp.tile([128, 1024], BF16, tag="ptb")
                                emit_exp(pt[:], s_ps[:])
                                last_direct = pi == len(prs) - 1
                                jobs = [
                                    (
                                        "direct", I, g, hh, 2 * pr, 2 * pr + 1,
                                        pt, last_direct,
                                    )
                                ]
                                if 1 <= C - I <= 2:
                                    mir_pend.setdefault(C, []).append(pt)
                                    if len(mir_pend[C]) == 2:
                                        pts_ = mir_pend.pop(C)
                                        jobs.append(("mirror", C, pts_, 0))
                                        jobs.append(("mirror", C, pts_, 1))
                                for job in jobs:
                                    pending.append(job)
                                while len(pending) > 6:
                                    run_job(pending.pop(0), ctx_mm, mirror_job)
                                if (g or I or pi) and deferred:
                                    deferred.pop(0)()
                            for job in pending:
                                run_job(job, ctx_mm, mirror_job)

                        # qg I fully accumulated (its mirrors arrived earlier)
                        def norm_qg(half=half, I=I, ctx_ps=get_ctx(I)):
                            ctx_sb = tsb.tile(
                                [128, QW], F32R, name=f"cs{half}_{I}",
                                tag=f"cs{I % 2}",
                            )
                            nc.vector.tensor_copy(ctx_sb[:], ctx_ps[:])
                            # norm matmuls reuse the just-freed ctx bank so
                            # the scores pipeline keeps both of its slots
                            nrm = cps.tile(
                                [128, QW], F32, name=f"nrm{half}_{I}",
                                tag=f"ctx{I % 3}",
                            )
                            nc.tensor.matmul(
                                nrm[0:4, :],
                                indg_sb[:, 4 * half : 4 * (half + 1)],
                                ctx_sb[:],
                                start=True,
                                stop=True,
                            )
                            r4 = tsb.tile([4, QW], F32R, tag="r4")
                            with nc.allow_low_precision(
                                reason="f32r output is full fp32 precision"
                            ):
                                nc.vector.reciprocal(r4[:], nrm[0:4, :])
                            nc.tensor.matmul(
                                nrm[:, :], indb_sb[half][:], r4[:],
                                start=True, stop=True,
                            )
                            cbb = tsb.tile(
                                [128, QW], BF16, name=f"cb{half}_{I}",
                                tag=f"cb{half}{I}", bufs=1,
                            )
                            ctx_bbs[(half, I)] = cbb
                            nc.vector.tensor_tensor(
                                cbb[:], ctx_sb[:], nrm[:, :], AluOpType.mult
                            )

                        deferred.append(norm_qg)

                        if half == 1:
                            def out_qg(I=I):
                                for qt in range(QW // 128):
                                    op = cps.tile(
                                        [128, QW], F32, name=f"op{I}_{qt}",
                                        tag=f"ctx{I % 3}",
                                    )
                                    for hf_ in range(2):
                                        nc.tensor.matmul(
                                            op[:, 0:128],
                                            ctx_bbs[(hf_, I)][
                                                :, 128 * qt : 128 * (qt + 1)
                                            ],
                                            woTs[hf_][:],
                                            start=(hf_ == 0),
                                            stop=(hf_ == 1),
                                        )
                                    o_sb = tsb.tile([128, 128], F32, tag="osb")
                                    nc.vector.tensor_copy(o_sb[:], op[:, 0:128])
                                    nc.sync.dma_start(
                                        out=y[
                                            QW * I + 128 * qt :
                                            QW * I + 128 * (qt + 1),
                                            :,
                                        ],
                                        in_=o_sb[:],
                                    )

                            deferred.append(out_qg)
                for job in deferred:
                    job()

    nc.compile()
    return nc


def _host_consts():
    indg = np.zeros((128, 8), np.float32)
    for h in range(2):
        for g in range(4):
            indg[32 * g + 16, 4 * h + g] = 1.0
    indb = np.zeros((8, 128), np.float32)
    for h in range(2):
        for g in range(4):
            indb[4 * h + g, 32 * g : 32 * g + 17] = 1.0
    e8 = np.zeros((8, 128), np.float32)
    for j in range(8):
        e8[j, 16 * j : 16 * (j + 1)] = 1.0
    return indg, indb, e8


def _make_runner(nc):
    """Build the jitted SPMD executable ONCE."""
    import jax
    import numpy as _np
    from jax.sharding import Mesh, PartitionSpec
    from jax.experimental.shard_map import shard_map
    import concourse.mybir as mybir
    from concourse import bass2jax

    bass2jax.install_neuronx_cc_hook()
    in_names, out_names, out_avals = [], [], []
    pname = nc.partition_id_tensor.name if nc.partition_id_tensor else None
    for alloc in nc.m.functions[0].allocations:
        if not isinstance(alloc, mybir.MemoryLocationSet):
            continue
        name = alloc.memorylocations[0].name
        if alloc.kind == "ExternalInput":
            if name != pname:
                in_names.append(name)
        elif alloc.kind == "ExternalOutput":
            out_names.append(name)
            out_avals.append(
                jax.core.ShapedArray(
                    tuple(alloc.tensor_shape), mybir.dt.np(alloc.dtype)
                )
            )
    n_params = len(in_names)
    all_names = in_names + out_names + ([pname] if pname else [])
    zero_shapes = [
        ((B * a.shape[0],) + tuple(a.shape[1:]), a.dtype) for a in out_avals
    ]

    def _body(*args):
        operands = list(args)
        if pname is not None:
            operands.append(bass2jax.partition_id_tensor())
        return tuple(
            bass2jax._bass_exec_p.bind(
                *operands,
                out_avals=tuple(out_avals),
                in_names=tuple(all_names),
                out_names=tuple(out_names),
                lowering_input_output_aliases=(),
                sim_require_finite=True,
                sim_require_nnan=True,
                nc=nc,
            )
        )

    devices = jax.devices()[:B]
    mesh = Mesh(_np.asarray(devices), ("core",))
    donate = tuple(range(n_params, n_params + len(out_names)))
    sharded = jax.jit(
        shard_map(
            _body,
            mesh=mesh,
            in_specs=(PartitionSpec("core"),) * (n_params + len(out_names)),
            out_specs=(PartitionSpec("core"),) * len(out_names),
            check_rep=False,
        ),
        donate_argnums=donate,
        keep_unused=True,
    )

    def run(in_maps):
        concat_in = [
            np.concatenate([np.asarray(m[name]) for m in in_maps], axis=0)
            for name in in_names
        ]
        zeros = [np.zeros(s, d) for s, d in zero_shapes]
        outs = sharded(*concat_in, *zeros)
        yv = np.asarray(outs[out_names.index("y")]).reshape(B, S, D)
        return yv

    return run


def kernel(x, W_k, W_q, W_v, W_o):
    if "nc" not in _CACHE:
        _CACHE["nc"] = _build()
    nc = _CACHE["nc"]

    indg, indb, e8 = _host_consts()
    wk = np.ascontiguousarray(np.asarray(W_k, dtype=np.float32))
    wo = np.ascontiguousarray(np.asarray(W_o, dtype=np.float32))
    xs = np.ascontiguousarray(np.asarray(x, dtype=np.float32))
    in_maps = [
        {"x": xs[b], "wk": wk, "wo": wo, "indg": indg, "indb": indb, "e8": e8}
        for b in range(B)
    ]
    try:
        if "runner" not in _CACHE:
            _CACHE["runner"] = _make_runner(nc)
        return _CACHE["runner"](in_maps)
    except Exception:
        _CACHE.pop("runner", None)
        from concourse.bass_utils import run_bass_kernel_spmd

        res = run_bass_kernel_spmd(nc, in_maps, core_ids=list(range(B)))
        return np.stack([res.results[b]["y"] for b in range(B)], axis=0)


# revision 49
# speedup vs baseline: 1.0062x; 1.0054x over previous
"""Trainium2 Bass kernel for nn_MultiHeadAttention_2259152798076 (V3).

Faithful to the reference (source bug included): Q = K = V = x @ W_k.T.
Data-parallel over batch B=8 -> one batch per NeuronCore.

Per-core algorithm (S=2048, 8 heads x d_k=16), with a SYMMETRIC softmax
shift so the P matrix stays transposable:
  h = x @ W_k.T;  m_j = |h_j|^2 / 4
  P''[k,q] = exp(<h_k,h_q>/4 - m_k - m_q)     (symmetric, in (0, 1])
  ctx^T = sum_k (h_k e^{m_k}) P''[k,q];  denom = sum_k e^{m_k} P''[k,q]
  (the e^{m_k} compensation uses the SAME stored fp8 bias values, so the
  key-side shift cancels exactly; the query-side shift is softmax-invariant)
  y = (ctx / denom) @ W_o.T

Speed structure (instruction cost: matmul = out-free-size x cycles/row;
fp8e4+DoubleRow = 0.5 c/r; exp = free-size x 0.83ns, Act engine only):
  * scores: hi-lo fp8 pair h ~= hq + r (both e4m3) in ONE DoubleRow matmul
    per (chunk, head): contraction [32p x 2] computes hq'hq + r'hq + hq'r
    + b_k + b_q (bias rides as extra contraction rows 16-19, split hi/lo
    fp8). 0.5 cycles/row with ~bf16 accuracy.
  * SYMMETRIC MIRRORING: P'' is symmetric, so for score supers (C, I) with
    1 <= C - I <= 2 the exp'd bf16 tiles computed during q-group I are
    PE-transposed (128x128 blocks) and accumulated into ctx[C] immediately;
    q-group C then skips those key-groups. exp tiles drop 256 -> 176
    (Act-engine busy 267us -> ~190us; Act is the hard bottleneck since
    TRN2 runs activations nowhere else and GPSIMD cannot read PSUM).
  * ctx: bf16 (K=128/instr), 32-row head bands, one PSUM bank per q-group,
    3 alive at once (bank tags J mod 3); ones-column -> e^{m_k} gives the
    denominators in the same accumulation.
  * PSUM budget (8 banks): 2x scores [128,1024] + 3x ctx + 1x mirror
    transpose staging; norm/out-proj matmuls reuse the just-freed ctx bank
    (same pool tag) so the scores pipeline keeps both of its slots.
  * scheduling: ctx/mirror jobs run through a depth-6 pending queue and
    norm/out-proj are deferred into the next score stream, so the in-order
    PE queue never stalls behind a pending exp.
  * setup: batched full-tensor DVE/Act/Pool ops + PE transposes build the
    fp8 score operands (element-step-2 fp8 transposes), the bf16 aug ctx
    tensor (g = h * e^m via an indicator-matmul dim-expansion of e^m), and
    the per-head |h|^2 biases (DVE square + segmented reduce).

Layouts:
  SC8[half] fp8 [128, 4*2048] viewed [p, v, s]; head g of half at partitions
  32g..32g+32. v0/v1 = scores lhsT i-slices, v2/v3 = rhs i-slices:
    v0: p<16: hq dims, p>=16: r dims                  (i=0 of lhsT)
    v1: p<16: hq, 1@16, 1@17, bk_hi@18, bk_lo@19      (i=1 of lhsT)
    v2: p<16: hq, p>=16: hq (replicated)              (i=0 of rhs)
    v3: p<16: r, bq_hi@16, bq_lo@17, 1@18, 1@19       (i=1 of rhs)
  Sum over (p, i) = hq'hq + r'hq + hq'r + b_k + b_q.
  vb16all bf16 [128, c(16) x hh(8) x 32]: 16 dims of h*e^m + e^m col + zero
  pads to 32 so every ctx matmul writes full 32-row bands (garbage rows are
  masked by the indicator broadcast in the normalization step).
"""

import numpy as np

B, S, D, H, DK = 8, 2048, 128, 8, 16
NCH = S // 128          # 16 k-chunks of 128
QW = 512
LAM = 3.0               # exp rescale: bias = -(|h|^2 + 4) + 4*LAM
_CACHE = {}

# exp engine balance: modeled ns per [128,1024] activation instruction
_ACT_NS = 1063.0
_POOL_NS = 1517.0


def _build():
    import concourse.bacc as bacc
    import concourse.mybir as mybir
    from concourse import masks
    from concourse.alu_op_type import AluOpType
    from concourse.bass import BassScalarEngine
    from concourse.tile import TileContext

    F32 = mybir.dt.float32
    F32R = mybir.dt.float32r
    BF16 = mybir.dt.bfloat16
    FP8 = mybir.dt.float8e4
    I32 = mybir.dt.int32
    EXP = mybir.ActivationFunctionType.Exp
    DR = mybir.MatmulPerfMode.DoubleRow
    AX_X = mybir.AxisListType.X

    nc = bacc.Bacc("TRN2", target_bir_lowering=False, debug=False, num_devices=8)

    x = nc.dram_tensor("x", [S, D], F32, kind="ExternalInput")
    wk = nc.dram_tensor("wk", [D, D], F32, kind="ExternalInput")
    wo = nc.dram_tensor("wo", [D, D], F32, kind="ExternalInput")
    indg = nc.dram_tensor("indg", [128, 8], F32, kind="ExternalInput")
    indb = nc.dram_tensor("indb", [8, 128], F32, kind="ExternalInput")
    e8d = nc.dram_tensor("e8", [8, 128], F32, kind="ExternalInput")
    y = nc.dram_tensor("y", [S, D], F32, kind="ExternalOutput")

    # deterministic greedy exp balancer: Act reads PSUM directly; Pool cannot
    # access PSUM on TRN2, so Pool-assigned tiles pay a DVE PSUM->SBUF copy.
    eng_t = {"act": 0.0, "pool": 0.0, "dve": 0.0}
    _DVE_CP_NS = 1420.0

    def dve_busy(ns):
        eng_t["dve"] += ns

    with TileContext(nc) as tc:
        with tc.tile_pool(name="persist", bufs=1) as sb:
            ident = sb.tile([128, 128], F32)
            masks.make_identity(nc, ident[:])
            ident8 = sb.tile([128, 128], FP8)
            nc.vector.tensor_copy(ident8[:], ident[:])
            identb = sb.tile([128, 128], BF16)
            nc.vector.tensor_copy(identb[:], ident[:])

            x_sb = sb.tile([128, NCH * 128], F32)
            nc.sync.dma_start(
                out=x_sb[:].rearrange("p (n m) -> p n m", m=128),
                in_=x.rearrange("(n p) m -> p n m", p=128),
            )
            wk_sb = sb.tile([128, 128], F32)
            wo_sb = sb.tile([128, 128], F32)
            indg_sb = sb.tile([128, 8], F32R)
            indb_sb = [sb.tile([4, 128], F32R, name=f"indb{h}") for h in range(2)]
            nc.sync.dma_start(out=wk_sb[:], in_=wk[:])
            nc.sync.dma_start(out=wo_sb[:], in_=wo[:])
            indg_f = sb.tile([128, 8], F32)
            indb_f = [sb.tile([4, 128], F32, name=f"indbf{h}") for h in range(2)]
            nc.sync.dma_start(out=indg_f[:], in_=indg[:])
            nc.vector.tensor_copy(indg_sb[:], indg_f[:])
            for h in range(2):
                nc.sync.dma_start(
                    out=indb_f[h][:], in_=indb[4 * h : 4 * (h + 1), :]
                )
                nc.vector.tensor_copy(indb_sb[h][:], indb_f[h][:])

            wkT = sb.tile([128, 128], BF16)
            xT = sb.tile([128, S], BF16)
            wos = [sb.tile([128, 128], F32, name=f"wos{h}") for h in range(2)]
            woTs = [sb.tile([128, 128], BF16, name=f"woTs{h}") for h in range(2)]

            # fp8 score operand tensors, [p, v(4), s(2048)]
            sc8 = [sb.tile([128, 4 * S], FP8, name=f"sc8_{h}") for h in range(2)]
            # ctx lhsT tensor: single big tensor, chunk-major
            vb16all = sb.tile([128, NCH * 8 * 32], BF16)
            hfull = sb.tile([128, S], F32)               # h natural [p, c, hh, 16]
            v8full = sb.tile([128, S], FP8)
            resfull = sb.tile([128, S], F32)
            vl8full = sb.tile([128, S], FP8)

            def pool_copy(out_ap, in_ap):
                from concourse.bass import BassVectorEngine
                BassVectorEngine.tensor_copy(nc.gpsimd, out_ap, in_ap)

            def act_copy(out_ap, in_ap):
                nc.scalar.activation(
                    out_ap, in_ap, mybir.ActivationFunctionType.Copy, 0.0, 1.0
                )

            with (
                tc.tile_pool(name="initps", bufs=2, space="PSUM") as ips,
                tc.tile_pool(name="initsb", bufs=2) as isb,
                tc.tile_pool(name="in8ps", bufs=2, space="PSUM") as tps,
            ):
                # weight transposes
                tp = ips.tile([128, 512], F32, tag="t")
                nc.tensor.transpose(tp[:, 0:128], wk_sb[:], ident[:])
                nc.vector.tensor_copy(wkT[:], tp[:, 0:128])
                for h in range(2):
                    # spread W_o columns, transpose -> row-spread bf16 W_o.T
                    nc.vector.memset(wos[h][:], 0.0)
                    nc.vector.tensor_copy(
                        wos[h][:].rearrange("p (g c) -> p g c", c=32)[:, :, 0:16],
                        wo_sb[:, 64 * h : 64 * (h + 1)].rearrange(
                            "p (g c) -> p g c", c=16
                        ),
                    )
                    tph = ips.tile([128, 512], F32, tag="t")
                    nc.tensor.transpose(tph[:, 0:128], wos[h][:], ident[:])
                    nc.vector.tensor_copy(woTs[h][:], tph[:, 0:128])

                # xT via PE transposes, 4 chunks per PSUM tile
                for q in range(4):
                    tpn = ips.tile([128, 512], F32, tag="t")
                    for i in range(4):
                        nc.tensor.transpose(
                            tpn[:, 128 * i : 128 * (i + 1)],
                            x_sb[:, 512 * q + 128 * i : 512 * q + 128 * (i + 1)],
                            ident[:],
                        )
                    nc.vector.tensor_copy(xT[:, 512 * q : 512 * (q + 1)], tpn[:])

                # h chunks -> hfull (4 chunks per PSUM tile)
                for q in range(4):
                    hp4 = ips.tile([128, 512], F32, tag="hp")
                    for i in range(4):
                        c = 4 * q + i
                        nc.tensor.matmul(
                            hp4[:, 128 * i : 128 * (i + 1)],
                            xT[:, 128 * c : 128 * (c + 1)],
                            wkT[:],
                            start=True,
                            stop=True,
                        )
                    act_copy(hfull[:, 512 * q : 512 * (q + 1)], hp4[:])

                # batched quantization chains (spread across Act/DVE/Pool)
                act_copy(v8full[:], hfull[:])
                nc.vector.tensor_tensor(
                    resfull[:], hfull[:], v8full[:], AluOpType.subtract
                )
                pool_copy(vl8full[:], resfull[:])

                # bias: b4 = 8 - |h|^2 (LAM=3), split hi/lo fp8; [128, (c hh)]
                hsq = isb.tile([128, S], F32, tag="hsq", bufs=1)
                nc.vector.tensor_tensor(hsq[:], hfull[:], hfull[:], AluOpType.mult)
                hsum = isb.tile([128, 128], F32, tag="hsum", bufs=1)
                nc.vector.tensor_reduce(
                    hsum[:],
                    hsq[:].rearrange("p (ch k) -> p ch k", k=16),
                    AX_X,
                    AluOpType.add,
                )
                b4 = isb.tile([128, 128], F32, tag="b4", bufs=1)
                nc.vector.tensor_scalar(
                    b4[:], hsum[:], -1.0, 0.0, AluOpType.mult,
                    AluOpType.add,
                )
                bhi = isb.tile([128, 128], FP8, tag="bhi", bufs=1)
                bres = isb.tile([128, 128], F32, tag="bres", bufs=1)
                blo = isb.tile([128, 128], FP8, tag="blo", bufs=1)
                act_copy(bhi[:], b4[:])
                nc.vector.tensor_tensor(bres[:], b4[:], bhi[:], AluOpType.subtract)
                pool_copy(blo[:], bres[:])

                # ctx aug tensor: batched zero/ones + data writes
                nc.vector.memset(vb16all[:].bitcast(I32), 0)
                v8w = v8full[:].bitcast(I32).rearrange(
                    "p (c hh w) -> p c hh w", c=NCH, hh=8
                )
                vlw = vl8full[:].bitcast(I32).rearrange(
                    "p (c hh w) -> p c hh w", c=NCH, hh=8
                )

                # interleaved natural tensor [p, c, half, v, slot(128)] + transposes
                itl = isb.tile([128, NCH * 2 * 4 * 128], FP8, tag="itl", bufs=1)
                ilv = itl[:].rearrange(
                    "p (c hf v g k) -> p c hf v g k", c=NCH, hf=2, v=4, g=4
                )
                ilw = itl[:].bitcast(I32).rearrange(
                    "p (c hf v g w) -> p c hf v g w", c=NCH, hf=2, v=4, g=4
                )
                bhiv = bhi[:].rearrange("p (c hh o) -> p c hh o", c=NCH, o=1)
                blov = blo[:].rearrange("p (c hh o) -> p c hh o", c=NCH, o=1)
                for hf in range(2):
                    hs = slice(4 * hf, 4 * (hf + 1))
                    s8 = v8w[:, :, hs, :]
                    sl = vlw[:, :, hs, :]
                    # v0 = [hq; r]
                    nc.vector.tensor_copy(ilw[:, :, hf, 0, :, 0:4], s8)
                    pool_copy(ilw[:, :, hf, 0, :, 4:8], sl)
                    # v1 (lhsT i1) = [hq; 1@16, 1@17, bk_hi@18, bk_lo@19, 0...]
                    nc.vector.tensor_copy(ilw[:, :, hf, 1, :, 0:4], s8)
                    nc.gpsimd.memset(ilw[:, :, hf, 1, :, 4:8], 0)
                    nc.gpsimd.memset(ilv[:, :, hf, 1, :, 16:18], 1.0)
                    nc.vector.tensor_copy(ilv[:, :, hf, 1, :, 18:19], bhiv[:, :, hs, :])
                    nc.vector.tensor_copy(ilv[:, :, hf, 1, :, 19:20], blov[:, :, hs, :])
                    # v2 = [hq; hq]
                    nc.vector.tensor_copy(ilw[:, :, hf, 2, :, 0:4], s8)
                    pool_copy(ilw[:, :, hf, 2, :, 4:8], s8)
                    # v3 (rhs i1) = [r; bq_hi@16, bq_lo@17, 1@18, 1@19, 0...]
                    nc.vector.tensor_copy(ilw[:, :, hf, 3, :, 0:4], sl)
                    nc.gpsimd.memset(ilw[:, :, hf, 3, :, 4:8], 0)
                    nc.vector.tensor_copy(ilv[:, :, hf, 3, :, 16:17], bhiv[:, :, hs, :])
                    nc.vector.tensor_copy(ilv[:, :, hf, 3, :, 17:18], blov[:, :, hs, :])
                    nc.gpsimd.memset(ilv[:, :, hf, 3, :, 18:20], 1.0)

                itf = itl[:].rearrange("p (b k) -> p b k", k=128)  # b = (c hf v)
                for c in range(NCH):
                    for hf in range(2):
                        # fp8 PE transpose writes with element step 2
                        tp8 = tps.tile([128, 1024], FP8, tag="tp8")
                        t2 = tp8[:].rearrange(
                            "p (v k two) -> p v k two", v=4, two=2
                        )
                        for v in range(4):
                            nc.tensor.transpose(
                                t2[:, v, :, 0:1],
                                itf[:, 8 * c + 4 * hf + v, :],
                                ident8[:],
                            )
                        cpf = act_copy if (c + NCH * hf) % 2 else nc.vector.tensor_copy
                        cpf(
                            sc8[hf][:].rearrange("p (v s) -> p v s", v=4)[
                                :, :, 128 * c : 128 * (c + 1)
                            ],
                            t2[:, :, :, 0],
                        )

                # per-key compensation em = exp(-0.25*(bhi+blo)) from the SAME
                # stored fp8 bias values (exact cancellation of the key-side
                # bias baked into every P'' tile); g = h * em (per pos, head)
                bsum = isb.tile([128, 128], F32, tag="bsum", bufs=1)
                nc.vector.tensor_tensor(bsum[:], bhi[:], blo[:], AluOpType.add)
                em = isb.tile([128, 128], F32, tag="em", bufs=1)
                nc.scalar.activation(em[:], bsum[:], EXP, 0.0, -0.25)
                # expand em over the 16 dims of each head: per chunk,
                # em16 = emT.T @ E8  (E8[j, d] = 1 iff d in head j)
                e8f = sb.tile([8, 128], F32)
                nc.sync.dma_start(out=e8f[:], in_=e8d[:])
                e8 = sb.tile([8, 128], BF16)
                nc.vector.tensor_copy(e8[:], e8f[:])
                em16 = sb.tile([128, S], F32)
                for q in range(4):
                    ep4 = ips.tile([128, 512], F32, tag="hp")
                    for i in range(4):
                        c = 4 * q + i
                        emt = ips.tile([8, 128], F32, tag="emt")
                        nc.tensor.transpose(
                            emt[:], em[:, 8 * c : 8 * (c + 1)], ident[:]
                        )
                        emts = isb.tile([8, 128], BF16, tag="emts")
                        nc.vector.tensor_copy(emts[:], emt[:])
                        nc.tensor.matmul(
                            ep4[:, 128 * i : 128 * (i + 1)],
                            emts[:],
                            e8[:],
                            start=True,
                            stop=True,
                        )
                    act_copy(em16[:, 512 * q : 512 * (q + 1)], ep4[:])
                gfull = sb.tile([128, S], F32)
                nc.vector.tensor_tensor(gfull[:], hfull[:], em16[:], AluOpType.mult)
                nc.vector.tensor_copy(
                    vb16all[:].rearrange("p (c hh k) -> p c hh k", c=NCH, hh=8)[
                        :, :, :, 16:17
                    ],
                    em[:].rearrange("p (c hh o) -> p c hh o", c=NCH, o=1),
                )
                act_copy(
                    vb16all[:].rearrange("p (c hh k) -> p c hh k", c=NCH, hh=8)[
                        :, :, :, 0:16
                    ],
                    gfull[:].rearrange("p (c hh k) -> p c hh k", c=NCH, hh=8),
                )

            # ---- main loop (symmetric-P mirroring) ----
            # During qg I we compute score supers (C, I) for key-groups C in
            # the direct set; supers with 1 <= C - I <= 2 are additionally
            # MIRRORED: their exp'd P'' tiles (symmetric, bias -m_k - m_q) are
            # PE-transposed and accumulated into ctx[C] immediately, so qg C
            # skips those key-groups entirely (exp count 256 -> 176 tiles).
            # ctx PSUM: one bank per qg, 3 alive at a time (tags J % 3).
            # norm / out-proj / mirror-transpose PSUM all borrow "s" slots.
            with (
                tc.tile_pool(name="sps", bufs=2, space="PSUM") as sps,
                tc.tile_pool(name="ctxps", bufs=1, space="PSUM") as cps,
                tc.tile_pool(name="mtps", bufs=1, space="PSUM") as mtp,
                tc.tile_pool(name="ptpool", bufs=12) as ptp,
                tc.tile_pool(name="mirsb", bufs=8) as msb,
                tc.tile_pool(name="tailsb", bufs=3) as tsb,
            ):
                def emit_exp(out_ap, in_ap):
                    nc.scalar.activation(out_ap, in_ap, EXP, 0.0, 0.25)

                sc8v = [
                    sc8[half][:].rearrange("p (v s) -> p v s", v=4)
                    for half in range(2)
                ]
                vbv = vb16all[:].rearrange(
                    "p (c w k) -> p c w k", c=NCH, w=8
                )
                # direct key-group sets per qg: C >= J plus unmirrored old
                # groups (J - C > 2); mirrored: 1 <= C - J <= 2
                direct = {
                    J: [C for C in range(4) if C >= J or J - C > 2]
                    for C_ in [0] for J in range(4)
                }
                def run_job(job, ctx_mm_, mirror_job_):
                    if job[0] == "direct":
                        _, I_, g_, hh_, ca, cb, pt_, last_ = job
                        ctx_mm_(I_, g_, hh_, ca, pt_[:, 0:QW], False)
                        ctx_mm_(I_, g_, hh_, cb, pt_[:, QW : 2 * QW], last_)
                    else:
                        _, C_, pts_, rnd_ = job
                        mirror_job_(C_, pts_, rnd_)

                deferred = []
                ctx_bbs = {}
                for half in range(2):
                    ctxb = {}
                    started = set()

                    def get_ctx(J, half=half, ctxb_=None):
                        if J not in ctxb:
                            ctxb[J] = cps.tile(
                                [128, QW], F32, name=f"ctx{half}_{J}",
                                tag=f"ctx{J % 3}",
                            )
                        return ctxb[J]

                    def ctx_mm(J, g, hh, chunk, rhs_ap, last):
                        key = (g, J)
                        st = key not in started
                        started.add(key)
                        nc.tensor.matmul(
                            get_ctx(J)[32 * g : 32 * (g + 1), :],
                            vbv[:, chunk, hh, :],
                            rhs_ap,
                            start=st,
                            stop=last,
                            tile_position=(0, 32 * g),
                            skip_group_check=True,
                        )

                    for I in range(4):
                        q0 = QW * I
                        for g in range(4):
                            half_, g_, I_ = half, g, I
                            hh = 4 * half + g
                            pending = []
                            mir_pend = {}

                            def mirror_job(C, pts, rnd, half=half, g=g, I=I,
                                           hh=hh):
                                # transpose half the pt tiles of super (C, I)
                                # into mirror tiles (keys 4I+b, queries qg C)
                                mt = mtp.tile([128, 1024], BF16, tag="mt")
                                for j in range(4):  # source chunk 4C+j
                                    pt_src = pts[j // 2]
                                    dc = j % 2
                                    for db in range(2):
                                        b = 2 * rnd + db
                                        nc.tensor.transpose(
                                            mt[:, 512 * db + 128 * j :
                                               512 * db + 128 * (j + 1)],
                                            pt_src[:, 512 * dc + 128 * b :
                                                   512 * dc + 128 * (b + 1)],
                                            identb[:],
                                        )
                                mir = msb.tile([128, 1024], BF16, tag="mir")
                                nc.vector.tensor_copy(mir[:], mt[:])
                                for db in range(2):
                                    b = 2 * rnd + db
                                    ctx_mm(
                                        C, g, hh, 4 * I + b,
                                        mir[:, 512 * db : 512 * (db + 1)], False,
                                    )

                            prs = [
                                2 * C + t for C in direct[I] for t in range(2)
                            ]
                            for pi, pr in enumerate(prs):
                                C = pr // 2
                                s_ps = sps.tile([128, 1024], F32, tag="s")
                                for dc in range(2):
                                    c = 2 * pr + dc
                                    nc.tensor.matmul(
                                        s_ps[:, 512 * dc : 512 * (dc + 1)],
                                        sc8v[half][
                                            32 * g : 32 * (g + 1),
                                            0:2,
                                            128 * c : 128 * (c + 1),
                                        ],
                                        sc8v[half][
                                            32 * g : 32 * (g + 1), 2:4,
                                            q0 : q0 + QW,
                                        ],
                                        start=True,
                                        stop=True,
                                        perf_mode=DR,
                                        tile_position=(32 * g, 0),
                                        skip_group_check=True,
                                    )
                                pt = ptp.tile([128, 1024], BF16, tag="ptb")
                                emit_exp(pt[:], s_ps[:])
                                last_direct = pi == len(prs) - 1
                                jobs = [
                                    (
                                        "direct", I, g, hh, 2 * pr, 2 * pr + 1,
                                        pt, last_direct,
                                    )
                                ]
                                if 1 <= C - I <= 2:
                                    mir_pend.setdefault(C, []).append(pt)
                                    if len(mir_pend[C]) == 2:
                                        pts_ = mir_pend.pop(C)
                                        jobs.append(("mirror", C, pts_, 0))
                                        jobs.append(("mirror", C, pts_, 1))
                                for job in jobs:
                                    pending.append(job)
                                while len(pending) > 6:
                                    run_job(pending.pop(0), ctx_mm, mirror_job)
                                if (g or I or pi) and deferred:
                                    deferred.pop(0)()
                            for job in pending:
                                run_job(job, ctx_mm, mirror_job)

                        # qg I fully accumulated (its mirrors arrived earlier)
                        def norm_qg(half=half, I=I, ctx_ps=get_ctx(I)):
                            ctx_sb = tsb.tile(
                                [128, QW], F32R, name=f"cs{half}_{I}",
                                tag=f"cs{I % 2}",
                            )
                            nc.vector.tensor_copy(ctx_sb[:], ctx_ps[:])
                            # norm matmuls reuse the just-freed ctx bank so
                            # the scores pipeline keeps both of its slots
                            nrm = cps.tile(
                                [128, QW], F32, name=f"nrm{half}_{I}",
                                tag=f"ctx{I % 3}",
                            )
                            nc.tensor.matmul(
                                nrm[0:4, :],
                                indg_sb[:, 4 * half : 4 * (half + 1)],
                                ctx_sb[:],
                                start=True,
                                stop=True,
                            )
                            r4 = tsb.tile([4, QW], F32R, tag="r4")
                            with nc.allow_low_precision(
                                reason="f32r output is full fp32 precision"
                            ):
                                nc.vector.reciprocal(r4[:], nrm[0:4, :])
                            nc.tensor.matmul(
                                nrm[:, :], indb_sb[half][:], r4[:],
                                start=True, stop=True,
                            )
                            cbb = tsb.tile(
                                [128, QW], BF16, name=f"cb{half}_{I}",
                                tag=f"cb{half}{I}", bufs=1,
                            )
                            ctx_bbs[(half, I)] = cbb
                            nc.vector.tensor_tensor(
                                cbb[:], ctx_sb[:], nrm[:, :], AluOpType.mult
                            )

                        deferred.append(norm_qg)

                        if half == 1:
                            def out_qg(I=I):
                                for qt in range(QW // 128):
                                    op = cps.tile(
                                        [128, QW], F32, name=f"op{I}_{qt}",
                                        tag=f"ctx{I % 3}",
                                    )
                                    for hf_ in range(2):
                                        nc.tensor.matmul(
                                            op[:, 0:128],
                                            ctx_bbs[(hf_, I)][
                                                :, 128 * qt : 128 * (qt + 1)
                                            ],
                                            woTs[hf_][:],
                                            start=(hf_ == 0),
                                            stop=(hf_ == 1),
                                        )
                                    o_sb = tsb.tile([128, 128], F32, tag="osb")
                                    nc.vector.tensor_copy(o_sb[:], op[:, 0:128])
                                    nc.sync.dma_start(
                                        out=y[
                                            QW * I + 128 * qt :
                                            QW * I + 128 * (qt + 1),
                                            :,
                                        ],
                                        in_=o_sb[:],
                                    )

                            deferred.append(out_qg)
                for job in deferred:
                    job()

    nc.compile()
    return nc


def _host_consts():
    indg = np.zeros((128, 8), np.float32)
    for h in range(2):
        for g in range(4):
            indg[32 * g + 16, 4 * h + g] = 1.0
    indb = np.zeros((8, 128), np.float32)
    for h in range(2):
        for g in range(4):
            indb[4 * h + g, 32 * g : 32 * g + 17] = 1.0
    e8 = np.zeros((8, 128), np.float32)
    for j in range(8):
        e8[j, 16 * j : 16 * (j + 1)] = 1.0
    return indg, indb, e8


def _make_runner(nc):
    """Build the jitted SPMD executable ONCE."""
    import jax
    import numpy as _np
    from jax.sharding import Mesh, PartitionSpec
    from jax.experimental.shard_map import shard_map
    import concourse.mybir as mybir
    from concourse import bass2jax

    bass2jax.install_neuronx_cc_hook()
    in_names, out_names, out_avals = [], [], []
    pname = nc.partition_id_tensor.name if nc.partition_id_tensor else None
    for alloc in nc.m.functions[0].allocations:
        if not isinstance(alloc, mybir.MemoryLocationSet):
            continue
        name = alloc.memorylocations[0].name
        if alloc.kind == "ExternalInput":
            if name != pname:
                in_names.append(name)
        elif alloc.kind == "ExternalOutput":
            out_names.append(name)
            out_avals.append(
                jax.core.ShapedArray(
                    tuple(alloc.tensor_shape), mybir.dt.np(alloc.dtype)
                )
            )
    n_params = len(in_names)
    all_names = in_names + out_names + ([pname] if pname else [])
    zero_shapes = [
        ((B * a.shape[0],) + tuple(a.shape[1:]), a.dtype) for a in out_avals
    ]

    def _body(*args):
        operands = list(args)
        if pname is not None:
            operands.append(bass2jax.partition_id_tensor())
        return tuple(
            bass2jax._bass_exec_p.bind(
                *operands,
                out_avals=tuple(out_avals),
                in_names=tuple(all_names),
                out_names=tuple(out_names),
                lowering_input_output_aliases=(),
                sim_require_finite=True,
                sim_require_nnan=True,
                nc=nc,
            )
        )

    devices = jax.devices()[:B]
    mesh = Mesh(_np.asarray(devices), ("core",))
    donate = tuple(range(n_params, n_params + len(out_names)))
    sharded = jax.jit(
        shard_map(
            _body,
            mesh=mesh,
            in_specs=(PartitionSpec("core"),) * (n_params + len(out_names)),
            out_specs=(PartitionSpec("core"),) * len(out_names),
            check_rep=False,
        ),
        donate_argnums=donate,
        keep_unused=True,
    )

    def run(in_maps):
        concat_in = [
            np.concatenate([np.asarray(m[name]) for m in in_maps], axis=0)
            for name in in_names
        ]
        zeros = [np.zeros(s, d) for s, d in zero_shapes]
        outs = sharded(*concat_in, *zeros)
        yv = np.asarray(outs[out_names.index("y")]).reshape(B, S, D)
        return yv

    return run


def kernel(x, W_k, W_q, W_v, W_o):
    if "nc" not in _CACHE:
        _CACHE["nc"] = _build()
    nc = _CACHE["nc"]

    indg, indb, e8 = _host_consts()
    wk = np.ascontiguousarray(np.asarray(W_k, dtype=np.float32))
    wo = np.ascontiguousarray(np.asarray(W_o, dtype=np.float32))
    xs = np.ascontiguousarray(np.asarray(x, dtype=np.float32))
    in_maps = [
        {"x": xs[b], "wk": wk, "wo": wo, "indg": indg, "indb": indb, "e8": e8}
        for b in range(B)
    ]
    try:
        if "runner" not in _CACHE:
            _CACHE["runner"] = _make_runner(nc)
        return _CACHE["runner"](in_maps)
    except Exception:
        _CACHE.pop("runner", None)
        from concourse.bass_utils import run_bass_kernel_spmd

        res = run_bass_kernel_spmd(nc, in_maps, core_ids=list(range(B)))
        return np.stack([res.results[b]["y"] for b in range(B)], axis=0)
